# revision 11
# baseline (speedup 1.0000x reference)
"""Bass/Tile TRN2 kernel for nn_CSA_DGCN (4-layer dense-GAT GNN).

Sharding: pure data-parallel over batch. 8 cores x 32 graphs each; full
inputs in, full output out (shard/gather on host in kernel()).

Core ideas (per core, everything SBUF-resident):
  * h kept feature-major [HD=128 part, 32*128 free] f32, ping-pong buffers.
  * exp(leaky_relu(s_i+t_j)) == max(exp(s_i+t_j), exp(.2 s_i + .2 t_j))
    == max(outer(e^t,e^s), outer(e^.2t,e^.2s)) -> rank-1 PE outer products,
    no [N,N] transcendentals. Mask+edge-bias folded into precomputed
    expB = mask * exp(edge_w) multiplied in fp16 on GPSIMD.
  * softmax denominators via indicator-column matmuls accumulating into one
    [8,512] PSUM tile; reciprocal on [128,32] (denoms transposed onto
    partitions); per-(head,node) broadcast via DVE-expand + transpose-matmul.
  * strong-out matmuls write head-pairs [32,128] at 32-aligned PSUM bases
    using zero-gapped fp16 hp copies (PSUM accumulation of the two heads).
  * LayerNorm: center via (I-1/HD) matmul, variance via ones-matmul,
    affine folded into FFN weights on host. BN folded into embedding;
    final LN affine + mean-pool folded into proj weights.
"""

import numpy as np
from contextlib import ExitStack

import concourse.bass as bass
import concourse.tile as tile
from concourse import bacc, mybir
from concourse.bass_utils import run_bass_kernel_spmd

F32 = mybir.dt.float32
F16 = mybir.dt.float16
AF = mybir.ActivationFunctionType
ALU = mybir.AluOpType

B, N, IN, HD, NH, L, OUT = 256, 128, 9, 128, 8, 4, 60
DH = HD // NH
ALPHA = 0.6
EPS = 1e-5
NCORES = 8
BL = B // NCORES
G = 4
NG = BL // G
GN = G * N   # 512


def build_nc(profile=False, debug_taps=False):
    nc = bacc.Bacc("TRN2", target_bir_lowering=False, debug=False,
                   num_devices=NCORES)

    def din(name, shape, dt=F32):
        return nc.dram_tensor(name, shape, dt, kind="ExternalInput").ap()

    x_t = din("x_t", [BL, IN, N])
    wc_d = din("wc", [IN, HD])
    bc_d = din("bc", [HD, 1])
    ablk_d = din("ablk", [HD, L * 16])
    wl_d = din("wl", [HD, L * HD])
    w1f_d = din("w1f", [HD, L * 2 * HD])
    b1f_d = din("b1f", [HD, 2 * L])
    w2_d = din("w2", [HD, 2 * L * HD])
    b2_d = din("b2", [HD, L])
    awt_d = din("awt", [N, N], F16)
    expb_d = din("expb", [N, 4 * N], F16)      # 4 identical head blocks
    cmat_d = din("cmat", [HD, HD])
    ident_d = din("ident", [N, N])
    ind_d = din("ind", [N, 64], F16)           # indicator cols for denom rows
    projw_d = din("projw", [HD, HD])
    projb_d = din("projb", [HD, 1])
    hw1_d = din("hw1", [HD, HD // 2])
    hb1_d = din("hb1", [HD // 2, 1])
    hw2_d = din("hw2", [HD // 2, OUT])
    hb2_d = din("hb2", [OUT, 1])
    out_d = nc.dram_tensor("out", [OUT, BL], F32, kind="ExternalOutput").ap()
    dbg = {}
    if debug_taps:
        for nm, shp, dt in [("dbg_emb", [HD, GN], F32), ("dbg_hpf", [HD, GN], F32),
                            ("dbg_hpn16", [N, GN], F16), ("dbg_e1", [16, GN], F16),
                            ("dbg_wt", [N, 4 * N], F16), ("dbg_dnr", [8, GN], F32),
                            ("dbg_rec", [N, 32], F32), ("dbg_rbca", [HD, GN], F32),
                            ("dbg_t2", [HD, GN], F32), ("dbg_t3", [HD, GN], F32),
                            ("dbg_gpr", [HD, GN], F32), ("dbg_z", [HD, GN], F32),
                            ("dbg_h1", [HD, GN], F32), ("dbg_ef", [1, 2 * 16 * N], F16)]:
            dbg[nm] = nc.dram_tensor(nm, shp, dt, kind="ExternalOutput").ap()

    def tap(nm, t):
        if debug_taps and nm in dbg and nm not in tapped:
            tapped.add(nm)
            nc.sync.dma_start(dbg[nm], t)
    tapped = set()

    with tile.TileContext(nc) as tc, ExitStack() as ctx:
        cst = ctx.enter_context(tc.tile_pool(name="cst", bufs=1))
        sb = ctx.enter_context(tc.tile_pool(name="sb", bufs=2))
        sbw = ctx.enter_context(tc.tile_pool(name="sbw", bufs=3))
        psq = ctx.enter_context(tc.tile_pool(name="psq", bufs=2, space="PSUM"))
        psa = ctx.enter_context(tc.tile_pool(name="psa", bufs=2, space="PSUM"))
        psw = ctx.enter_context(tc.tile_pool(name="psw", bufs=2, space="PSUM"))
        pss = ctx.enter_context(tc.tile_pool(name="pss", bufs=1, space="PSUM"))

        def load(dram, shape, dt=F32):
            t = cst.tile(shape, dt, tag=dram.tensor.name)
            nc.sync.dma_start(t[:], dram)
            return t

        wc_s = load(wc_d, [IN, HD])
        bc_s = load(bc_d, [HD, 1])
        ablk_s = load(ablk_d, [HD, L * 16])
        wl_s = load(wl_d, [HD, L * HD])
        w1f_s = load(w1f_d, [HD, L * 2 * HD])
        b1f_s = load(b1f_d, [HD, 2 * L])
        w2_s = load(w2_d, [HD, 2 * L * HD])
        b2_s = load(b2_d, [HD, L])
        awt_s = load(awt_d, [N, N], F16)
        expb_s = load(expb_d, [N, 4 * N], F16)
        cmat_s = load(cmat_d, [HD, HD])
        ident_s = load(ident_d, [N, N])
        ind_s = load(ind_d, [N, 64], F16)
        projw_s = load(projw_d, [HD, HD])
        projb_s = load(projb_d, [HD, 1])
        hw1_s = load(hw1_d, [HD, HD // 2])
        hb1_s = load(hb1_d, [HD // 2, 1])
        hw2_s = load(hw2_d, [HD // 2, OUT])
        hb2_s = load(hb2_d, [OUT, 1])

        h_a = cst.tile([HD, BL * N], F32, tag="h_a")
        h_b = cst.tile([HD, BL * N], F32, tag="h_b")
        hbar = cst.tile([HD, BL], F32, tag="hbar")

        MM = nc.tensor.matmul

        # ---- embedding ----
        for g in range(NG):
            xe = sb.tile([IN, GN], F32, tag="xe")
            for k in range(G):
                nc.sync.dma_start(xe[:, k * N:(k + 1) * N], x_t[g * G + k])
            pe = psw.tile([HD, GN], F32, tag="work")
            for k in range(G):
                MM(pe[:, k * N:(k + 1) * N], wc_s[:], xe[:, k * N:(k + 1) * N],
                   start=True, stop=True)
            nc.scalar.activation(h_a[:, g * GN:(g + 1) * GN], pe[:],
                                 AF.Relu, bias=bc_s[:])
            if g == 0:
                tap("dbg_emb", h_a[:, 0:GN])

        def layernorm_z(src_sbuf, c0, tag_pfx):
            """Returns z tile [HD, GN] = (x-mean)/sqrt(var+eps), x = src."""
            pc = psw.tile([HD, GN], F32, tag="work")
            for k in range(G):
                s_ = slice(k * N, (k + 1) * N)
                MM(pc[:, s_], cmat_s[:], src_sbuf[:, c0 + k * N:c0 + (k + 1) * N],
                   start=True, stop=True)
            gsq = sb.tile([HD, GN], F32, tag=tag_pfx + "gsq")
            nc.scalar.activation(gsq[:], pc[:], AF.Square)
            pv = pss.tile([N, G], F32, tag="sm")
            for k in range(G):
                MM(pv[:, k:k + 1], gsq[:, k * N:(k + 1) * N], ones_f[:],
                   start=True, stop=True)
            sqv = sb.tile([N, G], F32, tag=tag_pfx + "sqv")
            nc.scalar.activation(sqv[:], pv[:], AF.Sqrt, scale=1.0 / HD,
                                 bias=eps_f[:])
            rst = sb.tile([N, G], F32, tag=tag_pfx + "rst")
            nc.vector.reciprocal(rst[:], sqv[:])
            rxz = sb.tile([N, GN], F32, tag=tag_pfx + "rxz")
            a = rst[:]
            src_x = bass.AP(tensor=a.tensor, offset=a.offset,
                            ap=[a.ap[0], [1, G], [0, HD]])
            nc.vector.tensor_copy(
                rxz[:].rearrange("p (k m) -> p k m", k=G), src_x)
            prb = psw.tile([HD, GN], F32, tag="work")
            for k in range(G):
                MM(prb[:, k * N:(k + 1) * N], rxz[:, k * N:(k + 1) * N],
                   ident_s[:], start=True, stop=True)
            rbc = sb.tile([HD, GN], F32, tag=tag_pfx + "rbc")
            nc.scalar.activation(rbc[:], prb[:], AF.Copy)
            z = sb.tile([HD, GN], F32, tag=tag_pfx + "z")
            nc.vector.scalar_tensor_tensor(
                z[:], pc[:], 1.0, rbc[:], op0=ALU.mult, op1=ALU.mult)
            return z

        ones_f = cst.tile([HD, 1], F32, tag="ones_f")
        nc.gpsimd.memset(ones_f[:], 1.0)
        eps_f = cst.tile([HD, 1], F32, tag="eps_f")
        nc.gpsimd.memset(eps_f[:], EPS)

        # ---- layers ----
        h_in, h_out = h_a, h_b
        for l in range(L):
            wl_l = wl_s[:, l * HD:(l + 1) * HD]
            ablk_l = ablk_s[:, l * 16:(l + 1) * 16]
            for g in range(NG):
                c0 = g * GN
                # hp matmuls
                pf = psw.tile([HD, GN], F32, tag="work")
                pn = psa.tile([HD, GN], F32, tag="acc")
                for k in range(G):
                    s_ = slice(k * N, (k + 1) * N)
                    MM(pf[:, s_], wl_l, h_in[:, c0 + k * N:c0 + (k + 1) * N],
                       start=True, stop=True)
                for k in range(G):
                    s_ = slice(k * N, (k + 1) * N)
                    MM(pn[:, s_], h_in[:, c0 + k * N:c0 + (k + 1) * N], wl_l,
                       start=True, stop=True)
                hpf = sb.tile([HD, GN], F32, tag="hpf")
                nc.scalar.activation(hpf[:], pf[:], AF.Copy)
                hpn16 = sb.tile([N, GN], F16, tag="hpn16")
                nc.vector.tensor_copy(hpn16[:], pn[:])
                if l == 0 and g == 0:
                    tap("dbg_hpf", hpf[:])
                    tap("dbg_hpn16", hpn16[:])
                # zero-gapped fp16 node-major hp: per b,pair j:
                #   [hp_{2j} (16) | zeros(32) | hp_{2j+1} (16)]
                hpz = sb.tile([N, G * 256], F16, tag="hpz")
                nc.gpsimd.memset(hpz[:], 0.0)
                for k in range(G):
                    dstk = bass.AP(
                        tensor=hpz[:].tensor,
                        offset=hpz[:].offset + k * 256,
                        ap=[hpz[:].ap[0], [64, 4], [48, 2], [1, 16]])
                    nc.vector.tensor_copy(
                        dstk,
                        hpn16[:, k * N:(k + 1) * N]
                        .rearrange("p (j q d) -> p j q d", j=4, q=2))
                # s,t + exps
                pst = pss.tile([16, GN], F32, tag="sm")
                for k in range(G):
                    s_ = slice(k * N, (k + 1) * N)
                    MM(pst[:, s_], ablk_l, hpf[:, s_], start=True, stop=True)
                e1 = sb.tile([16, GN], F16, tag="e1")
                e2 = sb.tile([16, GN], F16, tag="e2")
                nc.scalar.activation(e1[:], pst[:], AF.Exp)
                nc.scalar.activation(e2[:], pst[:], AF.Exp, scale=0.2)
                if l == 0 and g == 0:
                    tap("dbg_e1", e1[:])

                pg = psa.tile([HD, GN], F32, tag="acc")
                pw2_ = psw.tile([HD, GN], F32, tag="work")
                pdn = pss.tile([8, GN], F32, tag="sm")
                for k in range(G):
                    bc0 = k * N
                    # flatten this batch's 16 exp rows to one partition
                    ef = sbw.tile([1, 2 * 16 * N], F16, tag="ef")
                    nc.sync.dma_start(
                        ef[0:1, 0:16 * N].rearrange("p (r n) -> p r n", r=16),
                        e1[:, bc0:bc0 + N])
                    nc.sync.dma_start(
                        ef[0:1, 16 * N:].rearrange("p (r n) -> p r n", r=16),
                        e2[:, bc0:bc0 + N])

                    if l == 0 and g == 0 and k == 0:
                        tap("dbg_ef", ef[:])

                    def erow(which, r):  # which: 0=e1, 1=e2; r: 0..15
                        o = which * 16 * N + r * N
                        return ef[0:1, o:o + N]

                    for half in range(2):      # even heads, odd heads
                        pP = psq.tile([N, 4 * N], F32, tag="pq")
                        pQ = psq.tile([N, 4 * N], F32, tag="pq")
                        for hh in range(4):
                            h = 2 * hh + half
                            MM(pP[:, hh * N:(hh + 1) * N], erow(0, 8 + h),
                               erow(0, h), start=True, stop=True)
                            MM(pQ[:, hh * N:(hh + 1) * N], erow(1, 8 + h),
                               erow(1, h), start=True, stop=True)
                        qs = sbw.tile([N, 4 * N], F16, tag="qs")
                        nc.scalar.activation(qs[:], pQ[:], AF.Copy)
                        w0 = sbw.tile([N, 4 * N], F16, tag="w0")
                        nc.vector.scalar_tensor_tensor(
                            w0[:], pP[:], 1.0, qs[:], op0=ALU.mult,
                            op1=ALU.max)
                        wt = sbw.tile([N, 4 * N], F16, tag="wt")
                        nc.gpsimd.tensor_tensor(wt[:], w0[:], expb_s[:],
                                                op=ALU.mult)
                        if l == 0 and g == 0 and k == 0 and half == 0:
                            tap("dbg_wt", wt[:])
                        # strong: head-pair mms accumulate into [32,128]
                        # blocks at 32-aligned psum bases; zero-gapped lhsT
                        # puts head 2j in rows 0-15, head 2j+1 in 16-31.
                        for hh in range(4):
                            h = 2 * hh + half
                            j = h // 2
                            o = k * 256 + j * 64 + (h % 2) * 32
                            MM(pg[32 * j:32 * j + 32, bc0:bc0 + N],
                               hpz[:, o:o + 32],
                               wt[:, hh * N:(hh + 1) * N],
                               start=(h % 2 == 0), stop=(h % 2 == 1),
                               tile_position=(0, 32 * j))
                        # denominators into pdn row (half*4 + k)
                        r = half * 4 + k
                        MM(pdn[:], ind_s[:, r * 8:(r + 1) * 8], wt[:],
                           start=(k == 0 and half == 0),
                           stop=(k == G - 1 and half == 1))
                    # weak: contract over nodes j -> lhsT node-major hp
                    MM(pw2_[:, bc0:bc0 + N], hpn16[:, bc0:bc0 + N], awt_s[:],
                       start=True, stop=True)
                # denom -> partitions, reciprocal
                dnr = sb.tile([8, GN], F32, tag="dnr")
                nc.scalar.activation(dnr[:], pdn[:], AF.Copy)
                if l == 0 and g == 0:
                    tap("dbg_dnr", dnr[:])
                pdt = pss.tile([N, 32], F32, tag="sm")
                for hh in range(4):
                    nc.tensor.transpose(pdt[:, hh * 8:(hh + 1) * 8],
                                        dnr[:, hh * N:(hh + 1) * N],
                                        ident_s[0:8, 0:8])
                rec = sb.tile([N, 32], F32, tag="rec")
                nc.vector.reciprocal(rec[:], pdt[:])
                if l == 0 and g == 0:
                    tap("dbg_rec", rec[:])
                # alpha/denom broadcast to [HD, N] per batch
                rx = sb.tile([N, GN], F32, tag="rx")
                for k in range(G):
                    a = rec[:, k:]
                    src_x = bass.AP(tensor=a.tensor, offset=a.offset,
                                    ap=[a.ap[0], [4, 8], [0, 16]])
                    nc.vector.tensor_scalar_mul(
                        rx[:, k * N:(k + 1) * N]
                        .rearrange("p (h d) -> p h d", h=8), src_x, ALPHA)
                prc = psw.tile([HD, GN], F32, tag="work")
                for k in range(G):
                    MM(prc[:, k * N:(k + 1) * N], rx[:, k * N:(k + 1) * N],
                       ident_s[:], start=True, stop=True)
                rbca = sb.tile([HD, GN], F32, tag="rbca")
                nc.scalar.activation(rbca[:], prc[:], AF.Copy)
                if l == 0 and g == 0:
                    tap("dbg_rbca", rbca[:])
                t2 = sb.tile([HD, GN], F32, tag="t2")
                nc.vector.scalar_tensor_tensor(
                    t2[:], pg[:], 1.0, rbca[:], op0=ALU.mult, op1=ALU.mult)
                t3 = sb.tile([HD, GN], F32, tag="t3")
                nc.vector.scalar_tensor_tensor(
                    t3[:], pw2_[:], 1.0, t2[:], op0=ALU.mult, op1=ALU.add)
                gpr = sb.tile([HD, GN], F32, tag="gpr")
                nc.gpsimd.tensor_scalar(gpr[:], t3[:], 0.0, None,
                                        op0=ALU.max)
                if l == 0 and g == 0:
                    tap("dbg_t2", t2[:])
                    tap("dbg_t3", t3[:])
                    tap("dbg_gpr", gpr[:])
                # LN -> z
                z = layernorm_z(gpr, 0, "ln")
                if l == 0 and g == 0:
                    tap("dbg_z", z[:])
                # FFN
                w1a = w1f_s[:, l * 2 * HD:l * 2 * HD + HD]
                w1b = w1f_s[:, l * 2 * HD + HD:(l + 1) * 2 * HD]
                p1a = psw.tile([HD, GN], F32, tag="work")
                p1b = psa.tile([HD, GN], F32, tag="acc")
                for k in range(G):
                    s_ = slice(k * N, (k + 1) * N)
                    MM(p1a[:, s_], w1a, z[:, s_], start=True, stop=True)
                for k in range(G):
                    s_ = slice(k * N, (k + 1) * N)
                    MM(p1b[:, s_], w1b, z[:, s_], start=True, stop=True)
                r1a = sb.tile([HD, GN], F32, tag="r1a")
                r1b = sb.tile([HD, GN], F32, tag="r1b")
                nc.scalar.activation(r1a[:], p1a[:], AF.Relu,
                                     bias=b1f_s[:, 2 * l:2 * l + 1])
                nc.scalar.activation(r1b[:], p1b[:], AF.Relu,
                                     bias=b1f_s[:, 2 * l + 1:2 * l + 2])
                po = psw.tile([HD, GN], F32, tag="work")
                w2a = w2_s[:, (2 * l) * HD:(2 * l + 1) * HD]
                w2b = w2_s[:, (2 * l + 1) * HD:(2 * l + 2) * HD]
                for k in range(G):
                    s_ = slice(k * N, (k + 1) * N)
                    MM(po[:, s_], w2a, r1a[:, s_], start=True, stop=False)
                    MM(po[:, s_], w2b, r1b[:, s_], start=False, stop=True)
                nc.vector.scalar_tensor_tensor(
                    h_out[:, c0:c0 + GN], po[:], b2_s[:, l:l + 1],
                    h_in[:, c0:c0 + GN], op0=ALU.add, op1=ALU.add)
                if l == 0 and g == 0:
                    tap("dbg_h1", h_out[:, 0:GN])
            h_in, h_out = h_out, h_in

        # ---- final LN + pool ----
        for g in range(NG):
            z = layernorm_z(h_in, g * GN, "fl")
            nc.vector.tensor_reduce(
                hbar[:, g * G:(g + 1) * G],
                z[:].rearrange("p (k n) -> p k n", k=G),
                axis=mybir.AxisListType.X, op=ALU.add)

        # ---- head ----
        pp = pss.tile([HD, BL], F32, tag="sm")
        MM(pp[:], projw_s[:], hbar[:], start=True, stop=True)
        s1 = sb.tile([HD, BL], F32, tag="s1")
        nc.vector.tensor_scalar_add(s1[:], pp[:], projb_s[:])
        pm = pss.tile([HD // 2, BL], F32, tag="sm")
        MM(pm[:], hw1_s[:], s1[:], start=True, stop=True)
        r1 = sb.tile([HD // 2, BL], F32, tag="r1h")
        nc.scalar.activation(r1[:], pm[:], AF.Relu, bias=hb1_s[:])
        pout = pss.tile([OUT, BL], F32, tag="sm")
        MM(pout[:], hw2_s[:], r1[:], start=True, stop=True)
        ofin = sb.tile([OUT, BL], F32, tag="ofin")
        nc.vector.tensor_scalar_add(ofin[:], pout[:], hb2_s[:])
        nc.sync.dma_start(out_d[:], ofin[:])

    nc.compile()
    return nc


def _prep_consts(conv_w, conv_b, bn_gamma, bn_beta, bn_mean, bn_var, W,
                 a_src, a_dst, ln_g, ln_b, ffn_w1, ffn_b1, ffn_w2, ffn_b2,
                 norm_g, norm_b, proj_w, proj_b, head_w1, head_b1, head_w2,
                 head_b2, strong_adj, weak_mask, weak_param, edge_weight_adj):
    f = np.float32
    conv_w, conv_b = np.asarray(conv_w, f), np.asarray(conv_b, f)
    scale = (np.asarray(bn_gamma, f) / np.sqrt(np.asarray(bn_var, f) + EPS))
    wc = (conv_w * scale[:, None]).T.astype(f)
    bc = (((conv_b - np.asarray(bn_mean, f)) * scale)
          + np.asarray(bn_beta, f)).astype(f)[:, None]
    wp = np.asarray(weak_param, f)
    aw = (1.0 / (1.0 + np.exp(-wp))) * np.asarray(weak_mask, f)
    aw = aw / (aw.sum(-1, keepdims=True) + EPS)
    awt = ((1.0 - ALPHA) * aw).T.astype(np.float16)
    expb = np.where(np.asarray(strong_adj, f) > 0,
                    np.exp(np.asarray(edge_weight_adj, f)), 0.0).T
    expb4 = np.tile(expb, (1, 4)).astype(np.float16)
    a_src, a_dst = np.asarray(a_src, f), np.asarray(a_dst, f)
    ablk = np.zeros((HD, L * 16), f)
    for l in range(L):
        for h in range(NH):
            ablk[h * DH:(h + 1) * DH, l * 16 + h] = a_src[l, h]
            ablk[h * DH:(h + 1) * DH, l * 16 + 8 + h] = a_dst[l, h]
    W = np.asarray(W, f)
    wl = np.concatenate([W[l] for l in range(L)], axis=1).astype(f)
    ln_g, ln_b = np.asarray(ln_g, f), np.asarray(ln_b, f)
    ffn_w1, ffn_b1 = np.asarray(ffn_w1, f), np.asarray(ffn_b1, f)
    ffn_w2, ffn_b2 = np.asarray(ffn_w2, f), np.asarray(ffn_b2, f)
    w1f = np.concatenate([ln_g[l][:, None] * ffn_w1[l] for l in range(L)],
                         axis=1).astype(f)
    b1f_full = np.stack([ffn_b1[l] + ln_b[l] @ ffn_w1[l] for l in range(L)],
                        axis=1).astype(f)          # [256, L]
    b1f = np.zeros((HD, 2 * L), f)
    for l in range(L):
        b1f[:, 2 * l] = b1f_full[0:HD, l]
        b1f[:, 2 * l + 1] = b1f_full[HD:2 * HD, l]
    w2 = np.concatenate(
        [np.concatenate([ffn_w2[l][0:HD], ffn_w2[l][HD:2 * HD]], axis=1)
         for l in range(L)], axis=1).astype(f)     # [128, 2*L*128]
    b2 = np.stack([ffn_b2[l] for l in range(L)], axis=1).astype(f)
    cmat = (np.eye(HD) - 1.0 / HD).astype(f)
    ident = np.eye(N, dtype=f)
    ind = np.zeros((N, 64), np.float16)
    for r in range(8):
        ind[:, r * 8 + r] = 1.0
    projw = ((np.asarray(norm_g, f)[:, None] * np.asarray(proj_w, f))
             / N).astype(f)
    projb = (np.asarray(norm_b, f) @ np.asarray(proj_w, f)
             + np.asarray(proj_b, f)).astype(f)[:, None]
    return dict(wc=wc, bc=bc, ablk=ablk, wl=wl, w1f=w1f, b1f=b1f, w2=w2,
                b2=b2, awt=awt, expb=expb4, cmat=cmat, ident=ident, ind=ind,
                projw=projw, projb=projb,
                hw1=np.asarray(head_w1, f),
                hb1=np.asarray(head_b1, f)[:, None],
                hw2=np.asarray(head_w2, f),
                hb2=np.asarray(head_b2, f)[:, None])


_NC_CACHE = None


def _run(x, consts, trace=False):
    global _NC_CACHE
    if _NC_CACHE is None:
        _NC_CACHE = build_nc()
    x = np.asarray(x, np.float32)
    in_maps = []
    for c in range(NCORES):
        m = dict(consts)
        m["x_t"] = np.ascontiguousarray(
            x[c * BL:(c + 1) * BL].transpose(0, 2, 1))
        in_maps.append(m)
    kw = {"trace": True} if trace else {}
    r = run_bass_kernel_spmd(_NC_CACHE, in_maps, list(range(NCORES)), **kw)
    out = np.concatenate([res["out"].T for res in r.results], axis=0)
    return out, r


def kernel(x, **w):
    consts = _prep_consts(**w)
    out, _ = _run(x, consts)
    return out
